# revision 15
# baseline (speedup 1.0000x reference)
"""Trainium2 Bass kernel for the ExpInstantaneousPhase loss.

Math:
    part1 + part2 = 2 + 2*(x*y + Hx*Hy)/(Ax*Ay)   (Ax^2 = x^2 + Hx^2)
    loss = -2*B - (2/N) * sum(e),  e = (x*y + Hx*Hy) * rsqrt((x^2+Hx^2)*(y^2+Hy^2))
where H is the Hilbert transform along the 3000-sample time axis. Both Hilberts
come from one complex FFT round-trip of z = x + i*y:
    a = IFFT(h * FFT(z)),  Hx = Im(a) - y,  Hy = x - Re(a)
The size-3000 FFT+mask+IFFT is factored (3000 = 120*25) into THREE batched
matmul stages on the tensor engine (twiddles and the h mask folded into the
per-group weight matrices):
    S1: per n2 in [0,25):  120x120 complex mats, contract n1      (bf16)
    S2: per k1 in [0,120): 25x25 complex mats D_k1 (h absorbed),
        contract n2; 5 k1 block-diag packed into 125x125          (fp8)
    S3: per m2 in [0,25):  120x120 complex mats, contract k1      (fp8)
Corner turns between stages are SBUF->SBUF DMAs in fp8.
Sharding: shot s -> core s (8 shots, 8 cores); 1200 signals/core.
"""

import numpy as np
import ml_dtypes

N = 3000
N1, N2 = 120, 25
S = 1200          # signals per core
C = 300           # signal-chunk width
NCH = S // C      # 4 chunks
NQ = 4            # elementwise sub-chunks per chunk
Q = C // NQ
NCORES = 8
BF = ml_dtypes.bfloat16
F8 = ml_dtypes.float8_e4m3

_CACHE = {}
TRACE = False
TRACE_KW = {}
STAGES = 99  # debug: 1=S1 only, 2=+turn1/S2, 3=+turn2/S3, 5=+elementwise, 6=full
REPEAT = 1   # bench: hardware-loop the whole computation this many times


def _build_weights():
    """Host-side: the three stages' lhsT weight stacks."""
    w = np.exp(-2j * np.pi / N)
    k1 = np.arange(N1)
    n2 = np.arange(N2)
    k2 = np.arange(N2)
    h = np.zeros(N)
    h[0] = 1.0
    h[N // 2] = 1.0
    h[1:N // 2] = 2.0

    # fp8 range management: scale S1 and S2 down 16x each, S3 up 256x.
    # Product is unchanged; S3's 1/3000 would be subnormal in fp8 otherwise.
    W1 = np.exp(-2j * np.pi * np.outer(k1, k1) / N1)        # [k1, n1]
    S1m = np.array([(1 / 16) * (w ** (j * k1))[:, None] * W1 for j in range(N2)])
    WN2 = np.exp(-2j * np.pi * np.outer(k2, n2) / N2)       # [k2, n2]
    WN2i = np.exp(+2j * np.pi * np.outer(n2, k2) / N2)      # [m2, k2]
    D = np.array([(1 / 16) * (WN2i @ (h[a + N1 * k2][:, None] * WN2)) for a in range(N1)])
    W1i = np.exp(+2j * np.pi * np.outer(k1, k1) / N1)       # [m1, k1]
    S3m = np.array([(256.0 / N) * W1i * (w ** (-j * k1))[None, :] for j in range(N2)])

    def lhst3(mat):  # [r, i, negi] lhsT stack from complex mat (out_dim, in_dim)
        t = mat.T  # lhsT = [contract, out]
        return np.stack([t.real, t.imag, -t.imag])

    w1 = np.stack([lhst3(S1m[j]) for j in range(N2)])       # [25, 3, 120, 120]
    w3 = np.stack([lhst3(S3m[j]) for j in range(N2)])       # [25, 3, 120, 120]

    # S2: block-diag pack of 5 k1 per group, k1 = 5*g + j  (j in [0,5))
    # input basis p = 25*j + n2 ; output basis q = 5*m2 + j
    w2 = np.zeros((24, 3, 125, 125))
    for g in range(24):
        blk = np.zeros((125, 125), dtype=complex)
        for j in range(5):
            for m2 in range(N2):
                blk[5 * m2 + j, 25 * j:25 * j + 25] = D[5 * g + j][m2, :]
        w2[g] = lhst3(blk)

    # S3 contraction basis r = 24*j + g  ->  k1 = 5*(r % 24) + r // 24
    perm = np.array([5 * (r % 24) + r // 24 for r in range(N1)])
    w3 = w3[:, :, perm, :]
    # pre-transpose to SBUF layout [contract_partition, group, part, out] so
    # the weight-load DMAs are fully contiguous per partition
    w1 = np.ascontiguousarray(w1.transpose(2, 0, 1, 3))   # [120, 25, 3, 120]
    w2 = np.ascontiguousarray(w2.transpose(2, 0, 1, 3))   # [125, 24, 3, 125]
    w3 = np.ascontiguousarray(w3.transpose(2, 0, 1, 3))   # [120, 25, 3, 120]
    return w1.astype(BF), w2.astype(F8), w3.astype(F8)


def _build_nc():
    import concourse.bacc as bacc
    import concourse.mybir as mybir
    from concourse.tile import TileContext
    from contextlib import nullcontext

    fp32 = mybir.dt.float32
    bf16 = mybir.dt.bfloat16
    fp8 = mybir.dt.float8e4
    ALU = mybir.AluOpType
    AF = mybir.ActivationFunctionType

    nc = bacc.Bacc(None, target_bir_lowering=False)
    x_d = nc.dram_tensor("x", [NCH, N1, N2, C], bf16, kind="ExternalInput")
    y_d = nc.dram_tensor("y", [NCH, N1, N2, C], bf16, kind="ExternalInput")
    w1_d = nc.dram_tensor("w1", [N1, N2, 3, N1], bf16, kind="ExternalInput")
    w2_d = nc.dram_tensor("w2", [125, 24, 3, 125], fp8, kind="ExternalInput")
    w3_d = nc.dram_tensor("w3", [N1, N2, 3, N1], fp8, kind="ExternalInput")
    acc_d = nc.dram_tensor("acc", [1, 450], fp32, kind="ExternalOutput")

    with TileContext(nc) as tc:
        with (
            tc.tile_pool(name="consts", bufs=1) as consts,
            tc.tile_pool(name="io", bufs=2) as io,
            tc.tile_pool(name="big", bufs=2) as big,
            tc.tile_pool(name="turn", bufs=2) as turn,
            tc.tile_pool(name="a3p", bufs=1) as a3p,
            tc.tile_pool(name="ew", bufs=1) as ew,
            tc.tile_pool(name="eap", bufs=1) as eap,
            tc.tile_pool(name="psum", bufs=3, space="PSUM") as psum,
            tc.tile_pool(name="psacc", bufs=1, space="PSUM") as psacc,
        ):
            # --- constants (loaded once, host pre-transposed => contiguous) ---
            w1_sb = consts.tile([N1, N2, 3, N1], bf16)
            nc.sync.dma_start(out=w1_sb, in_=w1_d[:, :, :, :])
            w2_sb = consts.tile([125, 24, 3, 125], fp8)
            nc.sync.dma_start(out=w2_sb, in_=w2_d[:, :, :, :])
            w3_sb = consts.tile([N1, N2, 3, N1], fp8)
            nc.sync.dma_start(out=w3_sb, in_=w3_d[:, :, :, :])
            ones_sb = consts.tile([N1, 1], bf16)
            nc.vector.memset(ones_sb, 1.0)

            acc_ps = psacc.tile([1, 512], fp32)

            x_r = x_d   # [NCH, 120, 25, C], host pre-chunked
            y_r = y_d

            env = dict(nc=nc, fp32=fp32, bf16=bf16, fp8=fp8, ALU=ALU, AF=AF,
                       io=io, big=big, turn=turn, a3p=a3p, ew=ew, psum=psum, eap=eap,
                       w1_sb=w1_sb, w2_sb=w2_sb, w3_sb=w3_sb, ones_sb=ones_sb,
                       acc_ps=acc_ps, acc_d=acc_d, x_r=x_r, y_r=y_r)
            rep_ctx = tc.For_i(0, REPEAT, 1) if REPEAT > 1 else nullcontext()
            with rep_ctx:
                _body(env)
            # evacuate the accumulator
            acc_sb = consts.tile([1, 450], fp32)
            if STAGES >= 6:
                nc.scalar.copy(out=acc_sb, in_=acc_ps[0:1, 0:450])
            else:
                nc.vector.memset(acc_sb, 0.0)
            nc.sync.dma_start(out=acc_d[:, :], in_=acc_sb)
    nc.finalize()
    return nc


def _body(env):
    nc = env["nc"]
    fp32, bf16, fp8, ALU, AF = env["fp32"], env["bf16"], env["fp8"], env["ALU"], env["AF"]
    io, big, turn, a3p, ew, psum = (env[k] for k in ("io", "big", "turn", "a3p", "ew", "psum"))
    eap = env["eap"]
    w1_sb, w2_sb, w3_sb, ones_sb = (env[k] for k in ("w1_sb", "w2_sb", "w3_sb", "ones_sb"))
    acc_ps, acc_d, x_r, y_r = (env[k] for k in ("acc_ps", "acc_d", "x_r", "y_r"))

    first_acc = [True]
    n_acc = NCH * NQ * ((N2 + 5) // 6)
    acc_i = [0]

    def load(ch):
        xb = io.tile([N1, N2, C], bf16, tag="xb")
        yb = io.tile([N1, N2, C], bf16, tag="yb")
        nc.sync.dma_start(out=xb, in_=x_r[ch, :, :, :])
        nc.sync.dma_start(out=yb, in_=y_r[ch, :, :, :])
        return xb, yb

    def s1(xb, yb):
        # A layout: [k1, n2, plane, c] so t1 can merge both planes per DMA
        A = big.tile([N1, N2, 2, C], fp8, tag="big")
        for j in range(N2):
            ps = psum.tile([128, 1024], fp32, tag="ps")
            pv = ps.rearrange("p (b w) -> p b w", b=2)
            wr = w1_sb[:, j, 0, :]
            wi = w1_sb[:, j, 1, :]
            wn = w1_sb[:, j, 2, :]
            nc.tensor.matmul(pv[:N1, 0, :C], wr, xb[:, j, :], start=True, stop=False)
            nc.tensor.matmul(pv[:N1, 1, :C], wr, yb[:, j, :], start=True, stop=False)
            nc.tensor.matmul(pv[:N1, 1, :C], wi, xb[:, j, :], start=False, stop=True)
            nc.tensor.matmul(pv[:N1, 0, :C], wn, yb[:, j, :], start=False, stop=True)
            nc.vector.tensor_copy(out=A[:, j, :, :], in_=pv[:N1, :, :C])
        return A

    def t1(A):
        # corner turn on SWDGE (gpsimd) to offload the Sync sequencer
        A2 = turn.tile([125, 24, 2, C], fp8, tag="turn")
        for g in range(24):
            nc.gpsimd.dma_start(out=A2[:, g, :, :], in_=A[5 * g:5 * g + 5, :, :, :])
        return A2

    def s2(A2):
        Csb = big.tile([125, 24, 2, C], fp8, tag="big")
        for g in range(24):
            ps = psum.tile([128, 1024], fp32, tag="ps")
            pv = ps.rearrange("p (b w) -> p b w", b=2)
            dr = w2_sb[:, g, 0, :]
            di = w2_sb[:, g, 1, :]
            dn = w2_sb[:, g, 2, :]
            nc.tensor.matmul(pv[:125, 0, :C], dr, A2[:, g, 0, :], start=True, stop=False)
            nc.tensor.matmul(pv[:125, 1, :C], dr, A2[:, g, 1, :], start=True, stop=False)
            nc.tensor.matmul(pv[:125, 1, :C], di, A2[:, g, 0, :], start=False, stop=True)
            nc.tensor.matmul(pv[:125, 0, :C], dn, A2[:, g, 1, :], start=False, stop=True)
            nc.scalar.copy(out=Csb[:, g, :, :], in_=pv[:125, :, :C])
        return Csb

    def t2(Csb):
        C2 = turn.tile([N1, N2, 2, C], fp8, tag="turn")
        for m2 in range(N2):
            nc.gpsimd.dma_start(out=C2[:, m2, :, :],
                                in_=Csb[5 * m2:5 * m2 + 5, :, :, :])
        return C2

    def s3(C2):
        a3 = a3p.tile([N1, 2, N2, C], bf16, tag="a3")
        for m2 in range(N2):
            ps = psum.tile([128, 1024], fp32, tag="ps")
            pv = ps.rearrange("p (b w) -> p b w", b=2)
            wr = w3_sb[:, m2, 0, :]
            wi = w3_sb[:, m2, 1, :]
            wn = w3_sb[:, m2, 2, :]
            nc.tensor.matmul(pv[:N1, 0, :C], wr, C2[:, m2, 0, :], start=True, stop=False)
            nc.tensor.matmul(pv[:N1, 1, :C], wr, C2[:, m2, 1, :], start=True, stop=False)
            nc.tensor.matmul(pv[:N1, 1, :C], wi, C2[:, m2, 0, :], start=False, stop=True)
            nc.tensor.matmul(pv[:N1, 0, :C], wn, C2[:, m2, 1, :], start=False, stop=True)
            nc.scalar.copy(out=a3[:, :, m2, :], in_=pv[:N1, :, :C])
        return a3

    # software pipeline, skewed so the PE never waits on a fresh corner turn:
    #   iter k: load(k+1), S2(k), T2(k), S1(k+1), T1(k+1), S3(k), EW(k)
    # running e-accumulator in SBUF (summed over all chunks/hh on DVE;
    # reduced across partitions by 5 PE matmuls at the very end)
    eacc = eap.tile([N1, N2, Q], bf16, tag="eacc")
    nc.vector.memset(eacc, 0.0)
    env["eacc"] = eacc

    xb, yb = load(0)
    A2 = t1(s1(xb, yb))
    nxt = None
    for ch in range(NCH):
        if ch + 1 < NCH:
            nxt = load(ch + 1)
        Csb = s2(A2)
        C2 = t2(Csb)
        if ch + 1 < NCH:
            A2 = t1(s1(*nxt))
        a3 = s3(C2)
        _ew(env, ch, xb, yb, a3, first_acc, acc_i, n_acc)
        if ch + 1 < NCH:
            xb, yb = nxt

    if STAGES >= 6:
        nblk = (N2 + 5) // 6
        for i, b0 in enumerate(range(0, N2, 6)):
            bw = min(6, N2 - b0)
            nc.tensor.matmul(acc_ps[0:1, 0:bw * Q], ones_sb,
                             eacc[:, b0:b0 + bw, :],
                             start=(i == 0), stop=(i == nblk - 1),
                             skip_group_check=True)


def _ew(env, ch, xb, yb, a3, first_acc, acc_i, n_acc):
    nc = env["nc"]
    bf16, AF = env["bf16"], env["AF"]
    ew = env["ew"]
    ones_sb, acc_ps = env["ones_sb"], env["acc_ps"]
    eacc = env["eacc"]
    if True:
        for hh in range(NQ):
            sl = slice(hh * Q, (hh + 1) * Q)
            xh = xb[:, :, sl]
            yh = yb[:, :, sl]
            arh = a3[:, 0, :, sl]
            aih = a3[:, 1, :, sl]
            hy = ew.tile([N1, N2, Q], bf16, tag="hy")
            hx = ew.tile([N1, N2, Q], bf16, tag="hx")
            nc.vector.tensor_sub(hy, xh, arh)
            nc.vector.tensor_sub(hx, aih, yh)
            tp = ew.tile([N1, N2, Q], bf16, tag="tp")
            tq = ew.tile([N1, N2, Q], bf16, tag="tq")
            nc.vector.tensor_mul(tp, xh, yh)
            nc.vector.tensor_mul(tq, hx, hy)
            nc.vector.tensor_add(tp, tp, tq)           # num = x*y + hx*hy
            t1 = ew.tile([N1, N2, Q], bf16, tag="t1")
            # u*v = num^2 + cross^2 (Lagrange identity for 2-vectors)
            nc.vector.tensor_mul(t1, xh, hy)
            nc.vector.tensor_mul(tq, hx, yh)
            nc.vector.tensor_sub(t1, t1, tq)           # cross = x*hy - hx*y
            nc.scalar.square(out=tq, in_=tp)           # hy/hx dead; reuse slots
            nc.scalar.square(out=hy, in_=t1)
            nc.vector.tensor_add(tq, tq, hy)           # w = num^2 + cross^2
            nc.scalar.activation(out=hy, in_=tq, func=AF.Abs_reciprocal_sqrt)
            if STAGES < 6:
                continue
            nc.vector.tensor_mul(tq, tp, hy)           # e = num * rsqrt(w)
            nc.vector.tensor_add(eacc, eacc, tq)       # running sum (|sum|<=16)


def kernel(x: np.ndarray, y: np.ndarray) -> np.ndarray:
    from concourse.bass_utils import run_bass_kernel_spmd

    if "nc" not in _CACHE:
        _CACHE["nc"] = _build_nc()
        _CACHE["w"] = _build_weights()
    nc = _CACHE["nc"]
    w1, w2, w3 = _CACHE["w"]

    def prep(a):
        # [3000, 1200] -> [NCH, 120, 25, C] so chunk loads are contiguous
        return np.ascontiguousarray(
            a.reshape(N1, N2, NCH, C).transpose(2, 0, 1, 3)).astype(BF)

    in_maps = []
    for c in range(NCORES):
        xc = prep(x[c].reshape(N, S))
        yc = prep(y[c].reshape(N, S))
        in_maps.append({"x": xc, "y": yc, "w1": w1, "w2": w2, "w3": w3})

    res = run_bass_kernel_spmd(nc, in_maps, list(range(NCORES)),
                               trace=TRACE, **TRACE_KW)
    _CACHE["last"] = res
    total = 0.0
    for c in range(NCORES):
        total += res.results[c]["acc"].astype(np.float64).sum()
    btot = NCORES * S
    loss = -2.0 * btot - (2.0 / N) * total
    return np.float32(loss)

